# revision 1
# baseline (speedup 1.0000x reference)
"""Trainium2 Bass kernel for nn_LocallyDense: gather -> 16 group-GEMMs -> Conv1D(k=32) -> LeakyReLU.

Strategy: data-parallel over batch (32 -> 4 per core on 8 cores).
Host staging: apply the idx permutation + transpose while sharding (pure data
movement), so the device sees dense GEMMs only:
  stage 1: h[b] = x_perm[b] @ W[g] per group, computed as hT[d, (b,pos)]
  stage 2: y[b] = LeakyReLU(conv1d(h[b])) as a K=8192 GEMM accumulated in PSUM.
All matmuls in float32r (full-rate at moving-dim >= 256, ~1.5e-4 rel err).
Device output layout y[b, o, t]; host transposes back to [b, t, o].
"""
import numpy as np

import concourse.bass as bass
import concourse.mybir as mybir
import concourse.tile as tile
from concourse.alu_op_type import AluOpType
from concourse import bacc
from concourse.bass_utils import run_bass_kernel_spmd

B, N, F, G, S, D = 32, 1024, 512, 16, 64, 256
KC, O = 32, 512            # conv kernel taps, conv out channels
T = N - KC + 1             # 993 valid conv outputs
NCORES = 8
BPC = B // NCORES          # batches per core
NEG_SLOPE = 0.2
F32 = mybir.dt.float32
F32R = mybir.dt.float32r

TRACE = False              # test.py flips this to get a profile
STAGES = (1, 2)            # bench knob: which stages to emit
_cache = {}


def _build():
    nc = bacc.Bacc("TRN2", target_bir_lowering=False, debug=False,
                   num_devices=NCORES)
    xpt_d = nc.dram_tensor("xpt", [BPC, F, N], F32, kind="ExternalInput").ap()
    w_d = nc.dram_tensor("w", [G, F, D], F32, kind="ExternalInput").ap()
    b_d = nc.dram_tensor("b", [G, D], F32, kind="ExternalInput").ap()
    cw_d = nc.dram_tensor("cw", [4, KC * 2, 128, 128], F32,
                          kind="ExternalInput").ap()
    cb_d = nc.dram_tensor("cb", [O], F32, kind="ExternalOutput"
                          if False else "ExternalInput").ap()
    y_d = nc.dram_tensor("y", [BPC, O, T], F32, kind="ExternalOutput").ap()

    FKT = F // 128           # 4 k-tiles over F
    KK = KC * 2              # 64 k-chunks over (tap, d-half)
    with tile.TileContext(nc) as tc:
        with tc.tile_pool(name="xpt", bufs=4) as p_xpt, \
             tc.tile_pool(name="wg", bufs=6) as p_w, \
             tc.tile_pool(name="ht", bufs=1) as p_ht, \
             tc.tile_pool(name="bias", bufs=1) as p_bias, \
             tc.tile_pool(name="cw", bufs=2) as p_cw, \
             tc.tile_pool(name="yout", bufs=8) as p_out:

            # biases: b[g, m*128+p] -> b_sb[p, g*2+m]; conv_b[m*128+p] -> cb_sb[p, m]
            b_sb = p_bias.tile([128, G * 2], F32)
            nc.sync.dma_start(b_sb[:], b_d.rearrange("g (m p) -> p (g m)", p=128))
            cb_sb = p_bias.tile([128, 4], F32)
            nc.sync.dma_start(cb_sb[:], cb_d.rearrange("(m p) -> p m", p=128))

            # x permuted+transposed: per f-ktile a [128, BPC*N] tile, b-major cols
            xpt_sb = []
            for kt in range(FKT):
                t = p_xpt.tile([128, BPC * N], F32R, tag="xpt")
                for bb in range(BPC):
                    nc.sync.dma_start(
                        t[:, bb * N:(bb + 1) * N],
                        xpt_d[bb, kt * 128:(kt + 1) * 128, :].bitcast(F32R))
                xpt_sb.append(t)

            # conv weights per o-tile m: [128, KK*128], chunk k holds lhsT
            cw_sb = []
            for m in range(4):
                t = p_cw.tile([128, KK * 128], F32R, tag="cw")
                nc.sync.dma_start(
                    t[:].rearrange("p (k o) -> p k o", k=KK),
                    cw_d[m].bitcast(F32R).rearrange("k p o -> p k o"))
                cw_sb.append(t)

            # hT[m]: [128, BPC*N] float32r, d-half m on partitions
            ht_sb = [p_ht.tile([128, BPC * N], F32R, tag=f"ht{m}", name=f"ht{m}")
                     for m in range(2)]

            # ---------------- stage 1: group GEMMs ----------------
            if 1 in STAGES:
              with tc.tile_pool(name="ps1", bufs=6, space="PSUM") as p_ps1:
                  for g in range(G):
                      w_sb = p_w.tile([128, FKT * D], F32R, tag="wg")
                      nc.sync.dma_start(
                          w_sb[:].rearrange("p (kt d) -> p kt d", kt=FKT),
                          w_d[g].bitcast(F32R).rearrange("(kt p) d -> p kt d", p=128))
                      for m in range(2):
                          ps = p_ps1.tile([128, BPC * S], F32, tag="ps1")
                          for kt in range(FKT):
                              rhs = xpt_sb[kt][:].rearrange(
                                  "p (b t) -> p b t", b=BPC)[:, :, g * S:(g + 1) * S]
                              nc.tensor.matmul(
                                  ps[:], w_sb[:, kt * D + m * 128: kt * D + (m + 1) * 128],
                                  rhs, start=(kt == 0), stop=(kt == FKT - 1))
                          dest = ht_sb[m][:].rearrange(
                              "p (b t) -> p b t", b=BPC)[:, :, g * S:(g + 1) * S]
                          # h = psum + bias[g, m-half] (copy + rounds to f32r)
                          nc.vector.tensor_scalar_add(
                              dest, ps[:], b_sb[:, g * 2 + m: g * 2 + m + 1])

            # ---------------- stage 2: conv as GEMM ----------------
            ntiles = []
            for bb in range(BPC):
                ntiles.append((bb, 0, 512))
                # fp32r ISA requires even element counts; overlap col 511
                ntiles.append((bb, 511, 482))
            if 2 not in STAGES:
                ntiles = []
            with tc.tile_pool(name="ps2", bufs=8, space="PSUM") as p_ps2:
                for m in range(4 if 2 in STAGES else 0):
                    pss = [p_ps2.tile([128, 512], F32, tag="ps2", name=f"ps2_{m}_{j}")
                           for j in range(len(ntiles))]
                    for k in range(KK):
                        tap, dh = k // 2, k % 2
                        lhsT = cw_sb[m][:, k * 128:(k + 1) * 128]
                        for j, (bb, t0, nn) in enumerate(ntiles):
                            rhs = ht_sb[dh][:, bb * N + tap + t0:
                                            bb * N + tap + t0 + nn]
                            nc.tensor.matmul(pss[j][:, :nn], lhsT, rhs,
                                             start=(k == 0), stop=(k == KK - 1))
                    for j, (bb, t0, nn) in enumerate(ntiles):
                        y_sb = p_out.tile([128, 512], F32, tag="yout")
                        nc.scalar.activation(
                            y_sb[:, :nn], pss[j][:, :nn],
                            mybir.ActivationFunctionType.Identity,
                            bias=cb_sb[:, m:m + 1])
                        nc.vector.scalar_tensor_tensor(
                            y_sb[:, :nn], y_sb[:, :nn], NEG_SLOPE, y_sb[:, :nn],
                            AluOpType.mult, AluOpType.max)
                        nc.sync.dma_start(
                            y_d[bb, m * 128:(m + 1) * 128, t0:t0 + nn],
                            y_sb[:, :nn])
    nc.compile()
    return nc


def kernel(x, idx, W, b, conv_w, conv_b):
    x = np.asarray(x); idx = np.asarray(idx); W = np.asarray(W)
    b = np.asarray(b); conv_w = np.asarray(conv_w); conv_b = np.asarray(conv_b)
    if "nc" not in _cache:
        _cache["nc"] = _build()
    nc = _cache["nc"]

    idx_flat = idx.reshape(-1).astype(np.int64)
    # permute + transpose: xpt[b, f, p] = x[b, idx_flat[p], f]
    xpt = np.ascontiguousarray(x[:, idx_flat, :].transpose(0, 2, 1))
    # conv_w[tap, d, o] -> cw[m, (tap, dh), p, o]
    cw = np.ascontiguousarray(
        conv_w.reshape(KC, 2, 128, 4, 128).transpose(3, 0, 1, 2, 4)
    ).reshape(4, KC * 2, 128, 128)
    W_c = np.ascontiguousarray(W)
    b_c = np.ascontiguousarray(b)
    cb_c = np.ascontiguousarray(conv_b)

    in_maps = []
    for c in range(NCORES):
        in_maps.append({
            "xpt": xpt[c * BPC:(c + 1) * BPC],
            "w": W_c, "b": b_c, "cw": cw, "cb": cb_c,
        })
    res = run_bass_kernel_spmd(nc, in_maps, core_ids=list(range(NCORES)),
                               trace=TRACE)
    if TRACE and res.exec_time_ns is not None:
        print(f"HW exec time: {res.exec_time_ns} ns")
        if res.instructions_and_trace is not None:
            print("trace:", res.instructions_and_trace[1])
    y = np.concatenate([r["y"] for r in res.results], axis=0)  # [B, O, T]
    return np.ascontiguousarray(y.transpose(0, 2, 1)).astype(np.float32)



# revision 6
# speedup vs baseline: 1.4559x; 1.4559x over previous
"""Trainium2 Bass kernel for nn_LocallyDense: gather -> 16 group-GEMMs -> Conv1D(k=32) -> LeakyReLU.

Strategy: data-parallel over batch (32 -> 4 per core on 8 cores).
Host staging applies the idx permutation + transpose so the device sees dense
GEMMs only.

  stage 1 (bf16): h[d, (n,b)] = sum_kt W[g]^T x_perm, per group g; the fp32
    PSUM result is split into h_hi (fp8e4m3) + h_lo (fp8e5m2) residual.
  stage 2 (fp8 DoubleRow): conv as GEMM over (tap, d). Each DoubleRow matmul
    contracts both 128-halves of d in one instruction. Three terms recover
    near-bf16 accuracy: h_hi*w_hi + h_lo*w_hi + h_hi*w_lo, with conv weights
    pre-scaled by 64 into the e4m3 range (hi) + e5m2 residual (lo); the 1/64
    is folded out on the host (LeakyReLU is positively homogeneous).
  epilogue: LeakyReLU on DVE, bf16 output, host transposes/unscales.

Group bias b and conv bias are folded into a host-precomputed per-position
bias table added before the activation (they are zero for this problem's
inputs, in which case the add is skipped entirely).
"""
import numpy as np
import ml_dtypes

import concourse.bass as bass
import concourse.mybir as mybir
import concourse.tile as tile
from concourse.alu_op_type import AluOpType
from concourse import bacc
from concourse.bass_utils import run_bass_kernel_spmd

B, N, F, G, S, D = 32, 1024, 512, 16, 64, 256
KC, O = 32, 512            # conv taps, conv out channels
T = N - KC + 1             # 993 valid conv outputs
NCORES = 8
BPC = B // NCORES          # batches per core
FKT = F // 128             # k-tiles over F
NEG_SLOPE = 0.2
WSCALE = 64.0              # conv-weight pre-scale into e4m3 range (power of 2)

F32 = mybir.dt.float32
BF16 = mybir.dt.bfloat16
E4 = mybir.dt.float8e4
E5 = mybir.dt.float8e5
E4np = ml_dtypes.float8_e4m3
E5np = ml_dtypes.float8_e5m2
BFnp = ml_dtypes.bfloat16

# conv output tiles: (t0, nn). First tile reads positions <= 511 only
# (groups 0-7), so it can start while groups 8-15 are still being computed.
JT = [(0, 481), (481, 512)]

TRACE = False              # test.py flips this to get a profile
STAGES = (1, 2)            # bisect knob: 1 = stage-1 pieces, 2 = conv pieces
PIECES = ("mm", "hi", "lo", "cmm", "act", "dma")   # bisect knob
_cache = {}


def _build(has_bias: bool):
    nc = bacc.Bacc("TRN2", target_bir_lowering=False, debug=False,
                   num_devices=NCORES)
    xp_d = nc.dram_tensor("xp", [FKT, 128, N * BPC], BF16,
                          kind="ExternalInput").ap()
    w_d = nc.dram_tensor("w", [G, 128, FKT * D], BF16,
                         kind="ExternalInput").ap()
    cwh_d = nc.dram_tensor("cwh", [4, 128, KC * 2 * 128], E4,
                           kind="ExternalInput").ap()
    cwl_d = nc.dram_tensor("cwl", [4, 128, KC * 2 * 128], E5,
                           kind="ExternalInput").ap()
    y_d = nc.dram_tensor("y", [BPC, O, T], BF16, kind="ExternalOutput").ap()
    if has_bias:
        bc_d = nc.dram_tensor("bc", [4, 128, T], F32, kind="ExternalInput").ap()

    with tile.TileContext(nc) as tc:
        with tc.tile_pool(name="x", bufs=4) as p_x, \
             tc.tile_pool(name="wg", bufs=16) as p_w, \
             tc.tile_pool(name="ht", bufs=1) as p_ht, \
             tc.tile_pool(name="cw", bufs=1) as p_cw, \
             tc.tile_pool(name="bias", bufs=1) as p_bias, \
             tc.tile_pool(name="yout", bufs=8) as p_out, \
             tc.tile_pool(name="ps1", bufs=4, space="PSUM") as p_ps1, \
             tc.tile_pool(name="ps2", bufs=4, space="PSUM") as p_ps2:

            # ---- tiles ----
            x_sb = [p_x.tile([128, N * BPC], BF16, tag="x", name=f"x{kt}")
                    for kt in range(FKT)]
            w_sb = [p_w.tile([128, FKT * D], BF16, tag="w", name=f"w{g}")
                    for g in range(G)]
            # h layout: col = dh*(N*BPC) + n*BPC + b  (group ranges contiguous)
            ht_hi = p_ht.tile([128, 2 * N * BPC], E4, tag="hh", name="ht_hi")
            ht_lo = p_ht.tile([128, 2 * N * BPC], E5, tag="hl", name="ht_lo")
            cwh_sb = [p_cw.tile([128, KC * 2 * 128], E4, tag=f"cwh{m}",
                                name=f"cwh{m}") for m in range(4)]
            cwl_sb = [p_cw.tile([128, KC * 2 * 128], E5, tag=f"cwl{m}",
                                name=f"cwl{m}") for m in range(4)]
            if has_bias:
                bc_sb = [p_bias.tile([128, T], F32, tag=f"bc{m}", name=f"bc{m}")
                         for m in range(4)]

            hh_w = ht_hi[:].rearrange("p (dh n b) -> p dh n b", dh=2, b=BPC)
            hl_w = ht_lo[:].rearrange("p (dh n b) -> p dh n b", dh=2, b=BPC)
            cwh_v = [t[:].rearrange("p (tap dh o) -> p tap dh o", tap=KC, dh=2)
                     for t in cwh_sb]
            cwl_v = [t[:].rearrange("p (tap dh o) -> p tap dh o", tap=KC, dh=2)
                     for t in cwl_sb]

            HALF = N * BPC // 2

            def stage1(g):
                if 1 not in STAGES:
                    return
                for m in range(2):
                    ps = p_ps1.tile([128, BPC * S], F32, tag="ps1", name=f"ps1_{g}_{m}")
                    for kt in range(FKT):
                        lhsT = w_sb[g][:, kt * D + m * 128: kt * D + (m + 1) * 128]
                        rhs = x_sb[kt][:, g * S * BPC:(g + 1) * S * BPC]
                        nc.tensor.matmul(ps[:], lhsT, rhs,
                                         start=(kt == 0), stop=(kt == FKT - 1))
                    psv = ps[:].rearrange("p (n b) -> p n b", b=BPC)
                    hi = hh_w[:, m, g * S:(g + 1) * S, :]
                    lo = hl_w[:, m, g * S:(g + 1) * S, :]
                    if "hi" in PIECES:
                        nc.scalar.copy(hi, psv)                   # rounds to e4m3
                    if "lo" in PIECES:
                        nc.vector.scalar_tensor_tensor(           # lo = ps - hi
                            lo, psv, 1.0, hi, AluOpType.mult, AluOpType.subtract)

            def conv_tile(mo, bb, t0, nn):
                if 2 not in STAGES:
                    return
                ps = p_ps2.tile([128, 512], F32, tag="ps2", name=f"ps2_{mo}_{bb}_{t0}")
                ops = []
                for hv, wv in ((hh_w, cwh_v), (hl_w, cwh_v), (hh_w, cwl_v)):
                    for tap in range(KC):
                        # rhs [p, dh(2), nn]: dh stride N*BPC, n stride BPC
                        rhs = hv[:, :, t0 + tap: t0 + tap + nn, bb]
                        ops.append((wv[mo][:, tap], rhs))
                if "cmm" in PIECES:
                    for i, (l, r) in enumerate(ops):
                        nc.tensor.matmul(ps[:, :nn], l, r, start=(i == 0),
                                         stop=(i == len(ops) - 1),
                                         perf_mode=mybir.MatmulPerfMode.DoubleRow)
                if has_bias:
                    nc.vector.tensor_tensor(ps[:, :nn], ps[:, :nn],
                                            bc_sb[mo][:, t0:t0 + nn],
                                            AluOpType.add)
                y_sb = p_out.tile([128, 512], BF16, tag="y", name=f"y_{mo}_{bb}_{t0}")
                if "act" in PIECES:
                    nc.scalar.copy(y_sb[:, :nn], ps[:, :nn])  # ACT: psum -> bf16
                    nc.vector.scalar_tensor_tensor(           # LeakyReLU in-place
                        y_sb[:, :nn], y_sb[:, :nn], NEG_SLOPE, y_sb[:, :nn],
                        AluOpType.mult, AluOpType.max)
                if "dma" in PIECES:
                    nc.sync.dma_start(y_d[bb, mo * 128:(mo + 1) * 128, t0:t0 + nn],
                                      y_sb[:, :nn])

            # ---- DMA + compute emission (order == DMA priority) ----
            for kt in range(FKT):                     # x first half (pos < 512)
                nc.sync.dma_start(x_sb[kt][:, :HALF], xp_d[kt, :, :HALF])
            for g in range(8):                        # stage-1 weights, first half
                nc.sync.dma_start(w_sb[g][:], w_d[g])
            nc.sync.dma_start(cwh_sb[0][:], cwh_d[0])
            nc.sync.dma_start(cwl_sb[0][:], cwl_d[0])
            for g in range(8, G):                     # remaining stage-1 weights
                nc.sync.dma_start(w_sb[g][:], w_d[g])
            for kt in range(FKT):                     # x second half
                nc.sync.dma_start(x_sb[kt][:, HALF:], xp_d[kt, :, HALF:])
            if has_bias:
                for m in range(4):
                    nc.sync.dma_start(bc_sb[m][:], bc_d[m])

            for g in range(8):                        # stage 1, groups 0-7
                stage1(g)

            t0, nn = JT[0]                            # conv-A (needs groups 0-7)
            for mo in range(4):
                if mo >= 1:
                    nc.sync.dma_start(cwh_sb[mo][:], cwh_d[mo])
                    nc.sync.dma_start(cwl_sb[mo][:], cwl_d[mo])
                for bb in range(BPC):
                    conv_tile(mo, bb, t0, nn)

            for g in range(8, G):                     # stage 1, groups 8-15
                stage1(g)

            t0, nn = JT[1]                            # conv-B (needs all groups)
            for mo in range(4):
                for bb in range(BPC):
                    conv_tile(mo, bb, t0, nn)

    nc.compile()
    return nc


def kernel(x, idx, W, b, conv_w, conv_b):
    x = np.asarray(x); idx = np.asarray(idx); W = np.asarray(W)
    b = np.asarray(b); conv_w = np.asarray(conv_w); conv_b = np.asarray(conv_b)
    has_bias = bool(np.any(b) or np.any(conv_b))
    key = ("nc", has_bias)
    if key not in _cache:
        _cache[key] = _build(has_bias)
        _cache["nc"] = _cache[key]   # for test.py's TimelineSim hook
    nc = _cache[key]

    idx_flat = idx.reshape(-1).astype(np.int64)
    # x permuted + transposed: xp[c][kt, p, n, b] = x[4c+b, idx_flat[n], 128kt+p]
    xg = x[:, idx_flat, :].astype(BFnp)                       # [B, N, F]
    xp = np.ascontiguousarray(
        xg.transpose(2, 1, 0).reshape(FKT, 128, N, NCORES, BPC)
        .transpose(3, 0, 1, 2, 4)).reshape(NCORES, FKT, 128, N * BPC)

    # stage-1 weights: wq[g, p, kt, d] = W[g, 128kt+p, d]
    wq = np.ascontiguousarray(
        W.astype(BFnp).reshape(G, FKT, 128, D).transpose(0, 2, 1, 3)
    ).reshape(G, 128, FKT * D)

    # conv weights scaled into e4m3 range + e5m2 residual:
    # cw[mo, p, tap, dh, o] = conv_w[tap, 128dh+p, 128mo+o] * 64
    cws = (conv_w * np.float32(WSCALE)).reshape(KC, 2, 128, 4, 128)
    cws = np.ascontiguousarray(cws.transpose(3, 2, 0, 1, 4))  # [mo,p,tap,dh,o]
    cwh = cws.astype(E4np)
    cwl = (cws - cwh.astype(np.float32)).astype(E5np)
    cwh = cwh.reshape(4, 128, KC * 2 * 128)
    cwl = cwl.reshape(4, 128, KC * 2 * 128)

    in_maps = []
    for c in range(NCORES):
        m = {"xp": xp[c], "w": wq, "cwh": cwh, "cwl": cwl}
        if has_bias:
            # bias_conv[t, o] = sum_tap b[g(t+tap)] @ conv_w[tap] + conv_b
            M = np.einsum('gd,tdo->gto', b, conv_w, optimize=True)  # [G, KC, O]
            P = np.concatenate([np.zeros((G, 1, O), np.float32),
                                np.cumsum(M, axis=1)], axis=1)      # [G, KC+1, O]
            t = np.arange(T)
            q, r = t >> 6, t & 63
            j1 = np.minimum(64 - r, KC)
            bc = P[q, j1] + (P[np.minimum(q + 1, G - 1), KC]
                             - P[np.minimum(q + 1, G - 1), j1]) * (j1 < KC)[:, None]
            bc = (bc + conv_b[None, :]) * np.float32(WSCALE)        # [T, O]
            m["bc"] = np.ascontiguousarray(
                bc.T.reshape(4, 128, T)).astype(np.float32)
        in_maps.append(m)

    res = run_bass_kernel_spmd(nc, in_maps, core_ids=list(range(NCORES)),
                               trace=TRACE)
    if TRACE and res.exec_time_ns is not None:
        print(f"HW exec time: {res.exec_time_ns} ns")
        if res.instructions_and_trace is not None:
            print("trace:", res.instructions_and_trace[1])
    y = np.stack([r["y"] for r in res.results])       # [NC, BPC, O, T] bf16
    y = y.reshape(B, O, T).transpose(0, 2, 1).astype(np.float32)
    return np.ascontiguousarray(y * np.float32(1.0 / WSCALE))


# revision 14
# speedup vs baseline: 1.5742x; 1.0812x over previous
"""Trainium2 Bass kernel for nn_LocallyDense: gather -> 16 group-GEMMs -> Conv1D(k=32) -> LeakyReLU.

Strategy: data-parallel over batch (32 -> 4 per core on 8 cores).
Host staging applies the idx permutation + transpose so the device sees dense
GEMMs only.

  stage 1 (bf16): h[d, (n,b)] = sum_kt W[g]^T x_perm, per group g; the fp32
    PSUM result is split into h_hi (fp8e4m3) + h_lo (fp8e5m2) residual.
  stage 2 (fp8 DoubleRow): conv as GEMM over (tap, d). Each DoubleRow matmul
    contracts both 128-halves of d in one instruction. Three terms recover
    near-bf16 accuracy: h_hi*w_hi + h_lo*w_hi + h_hi*w_lo, with conv weights
    pre-scaled by 64 into the e4m3 range (hi) + e5m2 residual (lo); the 1/64
    is folded out on the host (LeakyReLU is positively homogeneous).
  epilogue: LeakyReLU on DVE, bf16 output, host transposes/unscales.

Group bias b and conv bias are folded into a host-precomputed per-position
bias table added before the activation (they are zero for this problem's
inputs, in which case the add is skipped entirely).
"""
import numpy as np
import ml_dtypes

import concourse.bass as bass
import concourse.mybir as mybir
import concourse.tile as tile
from concourse.alu_op_type import AluOpType
from concourse import bacc
from concourse.bass_utils import run_bass_kernel_spmd

B, N, F, G, S, D = 32, 1024, 512, 16, 64, 256
KC, O = 32, 512            # conv taps, conv out channels
T = N - KC + 1             # 993 valid conv outputs
NCORES = 8
BPC = B // NCORES          # batches per core
FKT = F // 128             # k-tiles over F
NEG_SLOPE = 0.2
WSCALE = 64.0              # conv-weight pre-scale into e4m3 range (power of 2)

F32 = mybir.dt.float32
BF16 = mybir.dt.bfloat16
E4 = mybir.dt.float8e4
E5 = mybir.dt.float8e5
E4np = ml_dtypes.float8_e4m3
E5np = ml_dtypes.float8_e5m2
BFnp = ml_dtypes.bfloat16

# conv output tiles: (t0, nn). First tile reads positions <= 511 only
# (groups 0-7), so it can start while groups 8-15 are still being computed.
JT = [(0, 481), (481, 512)]
# taps that get the h_hi*w_lo correction term (the rest contribute ~1.3e-2
# of the 2e-2 error budget when dropped; each dropped tap saves ~3.3us)
CORR_TAPS = 24

TRACE = False              # test.py flips this to get a profile
STAGES = (1, 2)            # bisect knob: 1 = stage-1 pieces, 2 = conv pieces
PIECES = ("mm", "hi", "lo", "cmm", "act", "dma")   # bisect knob
_cache = {}


def _build(has_bias: bool):
    nc = bacc.Bacc("TRN2", target_bir_lowering=False, debug=False,
                   num_devices=NCORES)
    xp_d = nc.dram_tensor("xp", [FKT, 128, N * BPC], BF16,
                          kind="ExternalInput").ap()
    w_d = nc.dram_tensor("w", [4, 128, 4 * FKT * D], BF16,
                         kind="ExternalInput").ap()
    cwh_d = nc.dram_tensor("cwh", [4, 128, KC * 2 * 128], E4,
                           kind="ExternalInput").ap()
    cwl_d = nc.dram_tensor("cwl", [4, 128, KC * 2 * 128], E5,
                           kind="ExternalInput").ap()
    y_d = nc.dram_tensor("y", [BPC, O, T], BF16, kind="ExternalOutput").ap()
    if has_bias:
        bc_d = nc.dram_tensor("bc", [4, 128, T], F32, kind="ExternalInput").ap()

    with tile.TileContext(nc) as tc:
        with tc.tile_pool(name="x", bufs=4) as p_x, \
             tc.tile_pool(name="wg", bufs=4) as p_w, \
             tc.tile_pool(name="ht", bufs=1) as p_ht, \
             tc.tile_pool(name="cw", bufs=1) as p_cw, \
             tc.tile_pool(name="bias", bufs=1) as p_bias, \
             tc.tile_pool(name="yout", bufs=8) as p_out, \
             tc.tile_pool(name="ps1", bufs=4, space="PSUM") as p_ps1, \
             tc.tile_pool(name="ps2", bufs=4, space="PSUM") as p_ps2:

            # ---- tiles ----
            x_sb = [p_x.tile([128, N * BPC], BF16, tag="x", name=f"x{kt}")
                    for kt in range(FKT)]
            w_sb = [p_w.tile([128, 4 * FKT * D], BF16, tag="w", name=f"w{wt}")
                    for wt in range(4)]
            # h layout: col = dh*(N*BPC) + n*BPC + b  (group ranges contiguous)
            ht_hi = p_ht.tile([128, 2 * N * BPC], E4, tag="hh", name="ht_hi")
            ht_lo = p_ht.tile([128, 2 * N * BPC], E5, tag="hl", name="ht_lo")
            cwh_sb = [p_cw.tile([128, KC * 2 * 128], E4, tag=f"cwh{m}",
                                name=f"cwh{m}") for m in range(4)]
            cwl_sb = [p_cw.tile([128, KC * 2 * 128], E5, tag=f"cwl{m}",
                                name=f"cwl{m}") for m in range(4)]
            if has_bias:
                bc_sb = [p_bias.tile([128, T], F32, tag=f"bc{m}", name=f"bc{m}")
                         for m in range(4)]

            hh_w = ht_hi[:].rearrange("p (dh n b) -> p dh n b", dh=2, b=BPC)
            hl_w = ht_lo[:].rearrange("p (dh n b) -> p dh n b", dh=2, b=BPC)
            cwh_v = [t[:].rearrange("p (tap dh o) -> p tap dh o", tap=KC, dh=2)
                     for t in cwh_sb]
            cwl_v = [t[:].rearrange("p (tap dh o) -> p tap dh o", tap=KC, dh=2)
                     for t in cwl_sb]

            HALF = N * BPC // 2

            def stage1(g):
                if 1 not in STAGES:
                    return
                for m in range(2):
                    ps = p_ps1.tile([128, BPC * S], F32, tag="ps1", name=f"ps1_{g}_{m}")
                    for kt in range(FKT):
                        base = (g % 4) * FKT * D + kt * D + m * 128
                        lhsT = w_sb[g // 4][:, base: base + 128]
                        rhs = x_sb[kt][:, g * S * BPC:(g + 1) * S * BPC]
                        nc.tensor.matmul(ps[:], lhsT, rhs,
                                         start=(kt == 0), stop=(kt == FKT - 1))
                    psv = ps[:].rearrange("p (n b) -> p n b", b=BPC)
                    hi = hh_w[:, m, g * S:(g + 1) * S, :]
                    lo = hl_w[:, m, g * S:(g + 1) * S, :]
                    if "hi" in PIECES:
                        nc.scalar.copy(hi, psv)                   # rounds to e4m3
                    if "lo" in PIECES:
                        nc.vector.scalar_tensor_tensor(           # lo = ps - hi
                            lo, psv, 1.0, hi, AluOpType.mult, AluOpType.subtract)

            def conv_tile(mo, bb, t0, nn):
                if 2 not in STAGES:
                    return
                ps = p_ps2.tile([128, 512], F32, tag="ps2", name=f"ps2_{mo}_{bb}_{t0}")
                ops = []
                for hv, wv, ntap in ((hh_w, cwh_v, KC), (hl_w, cwh_v, KC),
                                     (hh_w, cwl_v, CORR_TAPS)):
                    for tap in range(ntap):
                        # rhs [p, dh(2), nn]: dh stride N*BPC, n stride BPC
                        rhs = hv[:, :, t0 + tap: t0 + tap + nn, bb]
                        ops.append((wv[mo][:, tap], rhs))
                if "cmm" in PIECES:
                    for i, (l, r) in enumerate(ops):
                        nc.tensor.matmul(ps[:, :nn], l, r, start=(i == 0),
                                         stop=(i == len(ops) - 1),
                                         perf_mode=mybir.MatmulPerfMode.DoubleRow)
                if has_bias:
                    nc.vector.tensor_tensor(ps[:, :nn], ps[:, :nn],
                                            bc_sb[mo][:, t0:t0 + nn],
                                            AluOpType.add)
                y_sb = p_out.tile([128, 512], BF16, tag="y", name=f"y_{mo}_{bb}_{t0}")
                if "act" in PIECES:
                    nc.scalar.copy(y_sb[:, :nn], ps[:, :nn])  # ACT: psum -> bf16
                    nc.vector.scalar_tensor_tensor(           # LeakyReLU in-place
                        y_sb[:, :nn], y_sb[:, :nn], NEG_SLOPE, y_sb[:, :nn],
                        AluOpType.mult, AluOpType.max)
                if "dma" in PIECES:
                    nc.sync.dma_start(y_d[bb, mo * 128:(mo + 1) * 128, t0:t0 + nn],
                                      y_sb[:, :nn])

            # ---- DMA + compute emission (order == DMA priority) ----
            Q = N * BPC // 4

            def xq(q):                                # x quarter (groups 4q..4q+3)
                for kt in range(FKT):
                    nc.sync.dma_start(x_sb[kt][:, q * Q:(q + 1) * Q],
                                      xp_d[kt, :, q * Q:(q + 1) * Q])

            xq(0)
            nc.sync.dma_start(w_sb[0][:], w_d[0])     # w groups 0-3
            nc.sync.dma_start(cwh_sb[0][:], cwh_d[0])
            xq(1)
            nc.sync.dma_start(w_sb[1][:], w_d[1])     # w groups 4-7
            nc.sync.dma_start(cwl_sb[0][:], cwl_d[0])
            xq(2)
            nc.sync.dma_start(w_sb[2][:], w_d[2])     # w groups 8-11
            xq(3)
            nc.sync.dma_start(w_sb[3][:], w_d[3])     # w groups 12-15
            for mo in range(1, 4):
                nc.sync.dma_start(cwh_sb[mo][:], cwh_d[mo])
                nc.sync.dma_start(cwl_sb[mo][:], cwl_d[mo])
            if has_bias:
                for m in range(4):
                    nc.sync.dma_start(bc_sb[m][:], bc_d[m])

            for g in range(8):                        # stage 1, groups 0-7
                stage1(g)

            t0, nn = JT[0]                            # conv-A (needs groups 0-7)
            for mo in range(4):
                for bb in range(BPC):
                    conv_tile(mo, bb, t0, nn)

            for g in range(8, G):                     # stage 1, groups 8-15
                stage1(g)

            t0, nn = JT[1]                            # conv-B (needs all groups)
            for mo in range(4):
                for bb in range(BPC):
                    if mo == 3 and bb == BPC - 1:
                        # split the final tile so the kernel tail (epilogue +
                        # store after the last matmul) is short
                        conv_tile(mo, bb, t0, 384)
                        conv_tile(mo, bb, t0 + 384, nn - 384)
                    else:
                        conv_tile(mo, bb, t0, nn)

    nc.compile()
    return nc


def kernel(x, idx, W, b, conv_w, conv_b):
    x = np.asarray(x); idx = np.asarray(idx); W = np.asarray(W)
    b = np.asarray(b); conv_w = np.asarray(conv_w); conv_b = np.asarray(conv_b)
    has_bias = bool(np.any(b) or np.any(conv_b))
    key = ("nc", has_bias)
    if key not in _cache:
        _cache[key] = _build(has_bias)
        _cache["nc"] = _cache[key]   # for test.py's TimelineSim hook
    nc = _cache[key]

    idx_flat = idx.reshape(-1).astype(np.int64)
    # x permuted + transposed: xp[c][kt, p, n, b] = x[4c+b, idx_flat[n], 128kt+p]
    xg = x[:, idx_flat, :].astype(BFnp)                       # [B, N, F]
    xp = np.ascontiguousarray(
        xg.transpose(2, 1, 0).reshape(FKT, 128, N, NCORES, BPC)
        .transpose(3, 0, 1, 2, 4)).reshape(NCORES, FKT, 128, N * BPC)

    # stage-1 weights, 4 groups per tile: wq[wt, p, gi, kt, d] = W[4wt+gi, 128kt+p, d]
    wq = np.ascontiguousarray(
        W.astype(BFnp).reshape(4, 4, FKT, 128, D).transpose(0, 3, 1, 2, 4)
    ).reshape(4, 128, 4 * FKT * D)

    # conv weights scaled into e4m3 range + e5m2 residual:
    # cw[mo, p, tap, dh, o] = conv_w[tap, 128dh+p, 128mo+o] * 64
    cws = (conv_w * np.float32(WSCALE)).reshape(KC, 2, 128, 4, 128)
    cws = np.ascontiguousarray(cws.transpose(3, 2, 0, 1, 4))  # [mo,p,tap,dh,o]
    cwh = cws.astype(E4np)
    cwl = (cws - cwh.astype(np.float32)).astype(E5np)
    cwh = cwh.reshape(4, 128, KC * 2 * 128)
    cwl = cwl.reshape(4, 128, KC * 2 * 128)

    in_maps = []
    for c in range(NCORES):
        m = {"xp": xp[c], "w": wq, "cwh": cwh, "cwl": cwl}
        if has_bias:
            # bias_conv[t, o] = sum_tap b[g(t+tap)] @ conv_w[tap] + conv_b
            M = np.einsum('gd,tdo->gto', b, conv_w, optimize=True)  # [G, KC, O]
            P = np.concatenate([np.zeros((G, 1, O), np.float32),
                                np.cumsum(M, axis=1)], axis=1)      # [G, KC+1, O]
            t = np.arange(T)
            q, r = t >> 6, t & 63
            j1 = np.minimum(64 - r, KC)
            bc = P[q, j1] + (P[np.minimum(q + 1, G - 1), KC]
                             - P[np.minimum(q + 1, G - 1), j1]) * (j1 < KC)[:, None]
            bc = (bc + conv_b[None, :]) * np.float32(WSCALE)        # [T, O]
            m["bc"] = np.ascontiguousarray(
                bc.T.reshape(4, 128, T)).astype(np.float32)
        in_maps.append(m)

    res = run_bass_kernel_spmd(nc, in_maps, core_ids=list(range(NCORES)),
                               trace=TRACE)
    if TRACE and res.exec_time_ns is not None:
        print(f"HW exec time: {res.exec_time_ns} ns")
        if res.instructions_and_trace is not None:
            print("trace:", res.instructions_and_trace[1])
    y = np.stack([r["y"] for r in res.results])       # [NC, BPC, O, T] bf16
    y = y.reshape(B, O, T).transpose(0, 2, 1).astype(np.float32)
    return np.ascontiguousarray(y * np.float32(1.0 / WSCALE))


# revision 18
# speedup vs baseline: 1.6064x; 1.0205x over previous
"""Trainium2 Bass kernel for nn_LocallyDense: gather -> 16 group-GEMMs -> Conv1D(k=32) -> LeakyReLU.

Strategy: data-parallel over batch (32 -> 4 per core on 8 cores).
Host staging applies the idx permutation + transpose so the device sees dense
GEMMs only.

  stage 1 (bf16): h[d, (n,b)] = sum_kt W[g]^T x_perm, per group g; the fp32
    PSUM result is split into h_hi (fp8e4m3) + h_lo (fp8e5m2) residual.
  stage 2 (fp8 DoubleRow): conv as GEMM over (tap, d). Each DoubleRow matmul
    contracts both 128-halves of d in one instruction. Three terms recover
    near-bf16 accuracy: h_hi*w_hi + h_lo*w_hi + h_hi*w_lo, with conv weights
    pre-scaled by 64 into the e4m3 range (hi) + e5m2 residual (lo); the 1/64
    is folded out on the host (LeakyReLU is positively homogeneous).
  epilogue: LeakyReLU on DVE, bf16 output, host transposes/unscales.

Group bias b and conv bias are folded into a host-precomputed per-position
bias table added before the activation (they are zero for this problem's
inputs, in which case the add is skipped entirely).
"""
import numpy as np
import ml_dtypes

import concourse.bass as bass
import concourse.mybir as mybir
import concourse.tile as tile
from concourse.alu_op_type import AluOpType
from concourse import bacc
from concourse.bass_utils import run_bass_kernel_spmd

B, N, F, G, S, D = 32, 1024, 512, 16, 64, 256
KC, O = 32, 512            # conv taps, conv out channels
T = N - KC + 1             # 993 valid conv outputs
NCORES = 8
BPC = B // NCORES          # batches per core
FKT = F // 128             # k-tiles over F
NEG_SLOPE = 0.2
WSCALE = 64.0              # conv-weight pre-scale into e4m3 range (power of 2)

F32 = mybir.dt.float32
BF16 = mybir.dt.bfloat16
E4 = mybir.dt.float8e4
E5 = mybir.dt.float8e5
E4np = ml_dtypes.float8_e4m3
E5np = ml_dtypes.float8_e5m2
BFnp = ml_dtypes.bfloat16

# conv output tiles: (t0, nn). First tile reads positions <= 511 only
# (groups 0-7), so it can start while groups 8-15 are still being computed.
JT = [(0, 481), (481, 512)]
# taps that get the h_hi*w_lo correction term (the rest contribute ~1.5e-2
# of the 2e-2 error budget when dropped; each dropped tap saves ~3.3us)
CORR_TAPS = 22

TRACE = False              # test.py flips this to get a profile
STAGES = (1, 2)            # bisect knob: 1 = stage-1 pieces, 2 = conv pieces
PIECES = ("mm", "hi", "lo", "cmm", "act", "dma")   # bisect knob
_cache = {}


def _build(has_bias: bool):
    nc = bacc.Bacc("TRN2", target_bir_lowering=False, debug=False,
                   num_devices=NCORES)
    xp_d = nc.dram_tensor("xp", [FKT, 128, N * BPC], BF16,
                          kind="ExternalInput").ap()
    w_d = nc.dram_tensor("w", [4, 128, 4 * FKT * D], BF16,
                         kind="ExternalInput").ap()
    cwh_d = nc.dram_tensor("cwh", [4, 128, KC * 2 * 128], E4,
                           kind="ExternalInput").ap()
    cwl_d = nc.dram_tensor("cwl", [4, 128, KC * 2 * 128], E5,
                           kind="ExternalInput").ap()
    y_d = nc.dram_tensor("y", [BPC, O, T], BF16, kind="ExternalOutput").ap()
    if has_bias:
        bc_d = nc.dram_tensor("bc", [4, 128, T], F32, kind="ExternalInput").ap()

    with tile.TileContext(nc) as tc:
        with tc.tile_pool(name="x", bufs=4) as p_x, \
             tc.tile_pool(name="wg", bufs=4) as p_w, \
             tc.tile_pool(name="ht", bufs=1) as p_ht, \
             tc.tile_pool(name="cw", bufs=1) as p_cw, \
             tc.tile_pool(name="bias", bufs=1) as p_bias, \
             tc.tile_pool(name="yout", bufs=8) as p_out, \
             tc.tile_pool(name="ps1", bufs=3, space="PSUM") as p_ps1, \
             tc.tile_pool(name="ps2", bufs=4, space="PSUM") as p_ps2:

            # ---- tiles ----
            x_sb = [p_x.tile([128, N * BPC], BF16, tag="x", name=f"x{kt}")
                    for kt in range(FKT)]
            w_sb = [p_w.tile([128, 4 * FKT * D], BF16, tag="w", name=f"w{wt}")
                    for wt in range(4)]
            # h layout: col = dh*(N*BPC) + n*BPC + b  (group ranges contiguous)
            ht_hi = p_ht.tile([128, 2 * N * BPC], E4, tag="hh", name="ht_hi")
            ht_lo = p_ht.tile([128, 2 * N * BPC], E5, tag="hl", name="ht_lo")
            cwh_sb = [p_cw.tile([128, KC * 2 * 128], E4, tag=f"cwh{m}",
                                name=f"cwh{m}") for m in range(4)]
            cwl_sb = [p_cw.tile([128, KC * 2 * 128], E5, tag=f"cwl{m}",
                                name=f"cwl{m}") for m in range(4)]
            if has_bias:
                bc_sb = [p_bias.tile([128, T], F32, tag=f"bc{m}", name=f"bc{m}")
                         for m in range(4)]

            hh_w = ht_hi[:].rearrange("p (dh n b) -> p dh n b", dh=2, b=BPC)
            hl_w = ht_lo[:].rearrange("p (dh n b) -> p dh n b", dh=2, b=BPC)
            cwh_v = [t[:].rearrange("p (tap dh o) -> p tap dh o", tap=KC, dh=2)
                     for t in cwh_sb]
            cwl_v = [t[:].rearrange("p (tap dh o) -> p tap dh o", tap=KC, dh=2)
                     for t in cwl_sb]

            HALF = N * BPC // 2

            def stage1(g):
                if 1 not in STAGES:
                    return
                for m in range(2):
                    ps = p_ps1.tile([128, BPC * S], F32, tag="ps1", name=f"ps1_{g}_{m}")
                    for kt in range(FKT):
                        base = (g % 4) * FKT * D + kt * D + m * 128
                        lhsT = w_sb[g // 4][:, base: base + 128]
                        rhs = x_sb[kt][:, g * S * BPC:(g + 1) * S * BPC]
                        nc.tensor.matmul(ps[:], lhsT, rhs,
                                         start=(kt == 0), stop=(kt == FKT - 1))
                    psv = ps[:].rearrange("p (n b) -> p n b", b=BPC)
                    hi = hh_w[:, m, g * S:(g + 1) * S, :]
                    lo = hl_w[:, m, g * S:(g + 1) * S, :]
                    if "hi" in PIECES:
                        nc.scalar.copy(hi, psv)                   # rounds to e4m3
                    if "lo" in PIECES:
                        nc.vector.scalar_tensor_tensor(           # lo = ps - hi
                            lo, psv, 1.0, hi, AluOpType.mult, AluOpType.subtract)

            def conv_tile(mo, bb, t0, nn):
                if 2 not in STAGES:
                    return
                ps = p_ps2.tile([128, 512], F32, tag="ps2", name=f"ps2_{mo}_{bb}_{t0}")
                ops = []
                for hv, wv, ntap in ((hh_w, cwh_v, KC), (hl_w, cwh_v, KC),
                                     (hh_w, cwl_v, CORR_TAPS)):
                    for tap in range(ntap):
                        # rhs [p, dh(2), nn]: dh stride N*BPC, n stride BPC
                        rhs = hv[:, :, t0 + tap: t0 + tap + nn, bb]
                        ops.append((wv[mo][:, tap], rhs))
                if "cmm" in PIECES:
                    for i, (l, r) in enumerate(ops):
                        nc.tensor.matmul(ps[:, :nn], l, r, start=(i == 0),
                                         stop=(i == len(ops) - 1),
                                         perf_mode=mybir.MatmulPerfMode.DoubleRow)
                if has_bias:
                    nc.vector.tensor_tensor(ps[:, :nn], ps[:, :nn],
                                            bc_sb[mo][:, t0:t0 + nn],
                                            AluOpType.add)
                y_sb = p_out.tile([128, 512], BF16, tag="y", name=f"y_{mo}_{bb}_{t0}")
                if "act" in PIECES:
                    nc.scalar.copy(y_sb[:, :nn], ps[:, :nn])  # ACT: psum -> bf16
                    nc.vector.scalar_tensor_tensor(           # LeakyReLU in-place
                        y_sb[:, :nn], y_sb[:, :nn], NEG_SLOPE, y_sb[:, :nn],
                        AluOpType.mult, AluOpType.max)
                if "dma" in PIECES:
                    nc.sync.dma_start(y_d[bb, mo * 128:(mo + 1) * 128, t0:t0 + nn],
                                      y_sb[:, :nn])

            # ---- DMA + compute emission (order == DMA priority) ----
            Q = N * BPC // 4

            def xq(q):                                # x quarter (groups 4q..4q+3)
                for kt in range(FKT):
                    nc.sync.dma_start(x_sb[kt][:, q * Q:(q + 1) * Q],
                                      xp_d[kt, :, q * Q:(q + 1) * Q])

            xq(0)
            nc.sync.dma_start(w_sb[0][:], w_d[0])     # w groups 0-3
            xq(1)
            nc.sync.dma_start(w_sb[1][:], w_d[1])     # w groups 4-7
            nc.sync.dma_start(cwh_sb[0][:], cwh_d[0])
            nc.sync.dma_start(cwl_sb[0][:], cwl_d[0])
            xq(2)
            nc.sync.dma_start(w_sb[2][:], w_d[2])     # w groups 8-11
            xq(3)
            nc.sync.dma_start(w_sb[3][:], w_d[3])     # w groups 12-15
            for mo in range(1, 4):
                nc.sync.dma_start(cwh_sb[mo][:], cwh_d[mo])
                nc.sync.dma_start(cwl_sb[mo][:], cwl_d[mo])
            if has_bias:
                for m in range(4):
                    nc.sync.dma_start(bc_sb[m][:], bc_d[m])

            # PE p-state warm-up: harmless tiny matmuls on a zeroed tile keep
            # the tensor engine busy (and its clock ramped) while the first
            # input DMAs land and across the stage-1 -> conv handoffs.
            warm_sb = p_x.tile([128, 64], BF16, tag="warm", name="warm_sb",
                               bufs=1)
            nc.vector.memset(warm_sb[:], 0)

            def pewarm(n, label):
                ps = p_ps1.tile([64, 64], F32, tag="warm", name=f"warm_{label}",
                                bufs=1)
                for i in range(n):
                    nc.tensor.matmul(ps[:], warm_sb[:, :64], warm_sb[:, :64],
                                     start=True, stop=True)

            pewarm(140, "boot")

            for g in range(8):                        # stage 1, groups 0-7
                stage1(g)

            pewarm(16, "bridge_a")                    # bridge the h-chain wait

            t0, nn = JT[0]                            # conv-A (needs groups 0-7)
            for mo in range(4):
                for bb in range(BPC):
                    conv_tile(mo, bb, t0, nn)

            for g in range(8, G):                     # stage 1, groups 8-15
                stage1(g)

            pewarm(16, "bridge_b")                    # bridge the h-chain wait

            t0, nn = JT[1]                            # conv-B (needs all groups)
            for mo in range(4):
                for bb in range(BPC):
                    if mo == 3 and bb == BPC - 1:
                        # split the final tile so the kernel tail (epilogue +
                        # store after the last matmul) is short
                        conv_tile(mo, bb, t0, 384)
                        conv_tile(mo, bb, t0 + 384, nn - 384)
                    else:
                        conv_tile(mo, bb, t0, nn)

    nc.compile()
    return nc


def kernel(x, idx, W, b, conv_w, conv_b):
    x = np.asarray(x); idx = np.asarray(idx); W = np.asarray(W)
    b = np.asarray(b); conv_w = np.asarray(conv_w); conv_b = np.asarray(conv_b)
    has_bias = bool(np.any(b) or np.any(conv_b))
    key = ("nc", has_bias)
    if key not in _cache:
        _cache[key] = _build(has_bias)
        _cache["nc"] = _cache[key]   # for test.py's TimelineSim hook
    nc = _cache[key]

    idx_flat = idx.reshape(-1).astype(np.int64)
    # x permuted + transposed: xp[c][kt, p, n, b] = x[4c+b, idx_flat[n], 128kt+p]
    xg = x[:, idx_flat, :].astype(BFnp)                       # [B, N, F]
    xp = np.ascontiguousarray(
        xg.transpose(2, 1, 0).reshape(FKT, 128, N, NCORES, BPC)
        .transpose(3, 0, 1, 2, 4)).reshape(NCORES, FKT, 128, N * BPC)

    # stage-1 weights, 4 groups per tile: wq[wt, p, gi, kt, d] = W[4wt+gi, 128kt+p, d]
    wq = np.ascontiguousarray(
        W.astype(BFnp).reshape(4, 4, FKT, 128, D).transpose(0, 3, 1, 2, 4)
    ).reshape(4, 128, 4 * FKT * D)

    # conv weights scaled into e4m3 range + e5m2 residual:
    # cw[mo, p, tap, dh, o] = conv_w[tap, 128dh+p, 128mo+o] * 64
    cws = (conv_w * np.float32(WSCALE)).reshape(KC, 2, 128, 4, 128)
    cws = np.ascontiguousarray(cws.transpose(3, 2, 0, 1, 4))  # [mo,p,tap,dh,o]
    cwh = cws.astype(E4np)
    cwl = (cws - cwh.astype(np.float32)).astype(E5np)
    cwh = cwh.reshape(4, 128, KC * 2 * 128)
    cwl = cwl.reshape(4, 128, KC * 2 * 128)

    in_maps = []
    for c in range(NCORES):
        m = {"xp": xp[c], "w": wq, "cwh": cwh, "cwl": cwl}
        if has_bias:
            # bias_conv[t, o] = sum_tap b[g(t+tap)] @ conv_w[tap] + conv_b
            M = np.einsum('gd,tdo->gto', b, conv_w, optimize=True)  # [G, KC, O]
            P = np.concatenate([np.zeros((G, 1, O), np.float32),
                                np.cumsum(M, axis=1)], axis=1)      # [G, KC+1, O]
            t = np.arange(T)
            q, r = t >> 6, t & 63
            j1 = np.minimum(64 - r, KC)
            bc = P[q, j1] + (P[np.minimum(q + 1, G - 1), KC]
                             - P[np.minimum(q + 1, G - 1), j1]) * (j1 < KC)[:, None]
            bc = (bc + conv_b[None, :]) * np.float32(WSCALE)        # [T, O]
            m["bc"] = np.ascontiguousarray(
                bc.T.reshape(4, 128, T)).astype(np.float32)
        in_maps.append(m)

    res = run_bass_kernel_spmd(nc, in_maps, core_ids=list(range(NCORES)),
                               trace=TRACE)
    if TRACE and res.exec_time_ns is not None:
        print(f"HW exec time: {res.exec_time_ns} ns")
        if res.instructions_and_trace is not None:
            print("trace:", res.instructions_and_trace[1])
    y = np.stack([r["y"] for r in res.results])       # [NC, BPC, O, T] bf16
    y = y.reshape(B, O, T).transpose(0, 2, 1).astype(np.float32)
    return np.ascontiguousarray(y * np.float32(1.0 / WSCALE))


# revision 22
# speedup vs baseline: 1.6218x; 1.0096x over previous
"""Trainium2 Bass kernel for nn_LocallyDense: gather -> 16 group-GEMMs -> Conv1D(k=32) -> LeakyReLU.

Strategy: data-parallel over batch (32 -> 4 per core on 8 cores).
Host staging applies the idx permutation + transpose so the device sees dense
GEMMs only.

  stage 1 (bf16): h[d, (n,b)] = sum_kt W[g]^T x_perm, per group g; the fp32
    PSUM result is split into h_hi (fp8e4m3) + h_lo (fp8e5m2) residual.
  stage 2 (fp8 DoubleRow): conv as GEMM over (tap, d). Each DoubleRow matmul
    contracts both 128-halves of d in one instruction. Three terms recover
    near-bf16 accuracy: h_hi*w_hi + h_lo*w_hi + h_hi*w_lo, with conv weights
    pre-scaled by 64 into the e4m3 range (hi) + e5m2 residual (lo); the 1/64
    is folded out on the host (LeakyReLU is positively homogeneous).
  epilogue: LeakyReLU on DVE, bf16 output, host transposes/unscales.

Group bias b and conv bias are folded into a host-precomputed per-position
bias table added before the activation (they are zero for this problem's
inputs, in which case the add is skipped entirely).
"""
import numpy as np
import ml_dtypes

import concourse.bass as bass
import concourse.mybir as mybir
import concourse.tile as tile
from concourse.alu_op_type import AluOpType
from concourse import bacc
from concourse.bass_utils import run_bass_kernel_spmd

B, N, F, G, S, D = 32, 1024, 512, 16, 64, 256
KC, O = 32, 512            # conv taps, conv out channels
T = N - KC + 1             # 993 valid conv outputs
NCORES = 8
BPC = B // NCORES          # batches per core
FKT = F // 128             # k-tiles over F
NEG_SLOPE = 0.2
WSCALE = 64.0              # conv-weight pre-scale into e4m3 range (power of 2)

F32 = mybir.dt.float32
BF16 = mybir.dt.bfloat16
E4 = mybir.dt.float8e4
E5 = mybir.dt.float8e5
E4np = ml_dtypes.float8_e4m3
E5np = ml_dtypes.float8_e5m2
BFnp = ml_dtypes.bfloat16

# conv output tiles: (t0, nn). First tile reads positions <= 511 only
# (groups 0-7), so it can start while groups 8-15 are still being computed.
JT = [(0, 481), (481, 512)]
# taps that get the h_hi*w_lo correction term (the rest contribute ~1.5e-2
# of the 2e-2 error budget when dropped; each dropped tap saves ~3.3us)
CORR_TAPS = 22

TRACE = False              # test.py flips this to get a profile
STAGES = (1, 2)            # bisect knob: 1 = stage-1 pieces, 2 = conv pieces
PIECES = ("mm", "hi", "lo", "cmm", "act", "dma")   # bisect knob
_cache = {}


def _build(has_bias: bool):
    nc = bacc.Bacc("TRN2", target_bir_lowering=False, debug=False,
                   num_devices=NCORES)
    xp_d = nc.dram_tensor("xp", [FKT, 128, N * BPC], BF16,
                          kind="ExternalInput").ap()
    w_d = nc.dram_tensor("w", [4, 128, 4 * FKT * D], BF16,
                         kind="ExternalInput").ap()
    cwh_d = nc.dram_tensor("cwh", [4, 128, KC * 2 * 128], E4,
                           kind="ExternalInput").ap()
    cwl_d = nc.dram_tensor("cwl", [4, 128, KC * 2 * 128], E5,
                           kind="ExternalInput").ap()
    y_d = nc.dram_tensor("y", [BPC, O, T], BF16, kind="ExternalOutput").ap()
    if has_bias:
        bc_d = nc.dram_tensor("bc", [4, 128, T], F32, kind="ExternalInput").ap()

    with tile.TileContext(nc) as tc:
        with tc.tile_pool(name="x", bufs=4) as p_x, \
             tc.tile_pool(name="wg", bufs=4) as p_w, \
             tc.tile_pool(name="ht", bufs=1) as p_ht, \
             tc.tile_pool(name="cw", bufs=1) as p_cw, \
             tc.tile_pool(name="bias", bufs=1) as p_bias, \
             tc.tile_pool(name="yout", bufs=8) as p_out, \
             tc.tile_pool(name="ps1", bufs=4, space="PSUM") as p_ps1, \
             tc.tile_pool(name="ps2", bufs=4, space="PSUM") as p_ps2:

            # ---- tiles ----
            x_sb = [p_x.tile([128, N * BPC], BF16, tag="x", name=f"x{kt}")
                    for kt in range(FKT)]
            w_sb = [p_w.tile([128, 4 * FKT * D], BF16, tag="w", name=f"w{wt}")
                    for wt in range(4)]
            # h layout: col = dh*(N*BPC) + n*BPC + b  (group ranges contiguous)
            ht_hi = p_ht.tile([128, 2 * N * BPC], E4, tag="hh", name="ht_hi")
            ht_lo = p_ht.tile([128, 2 * N * BPC], E5, tag="hl", name="ht_lo")
            cwh_sb = [p_cw.tile([128, KC * 2 * 128], E4, tag=f"cwh{m}",
                                name=f"cwh{m}") for m in range(4)]
            cwl_sb = [p_cw.tile([128, KC * 2 * 128], E5, tag=f"cwl{m}",
                                name=f"cwl{m}") for m in range(4)]
            if has_bias:
                bc_sb = [p_bias.tile([128, T], F32, tag=f"bc{m}", name=f"bc{m}")
                         for m in range(4)]

            hh_w = ht_hi[:].rearrange("p (dh n b) -> p dh n b", dh=2, b=BPC)
            hl_w = ht_lo[:].rearrange("p (dh n b) -> p dh n b", dh=2, b=BPC)
            cwh_v = [t[:].rearrange("p (tap dh o) -> p tap dh o", tap=KC, dh=2)
                     for t in cwh_sb]
            cwl_v = [t[:].rearrange("p (tap dh o) -> p tap dh o", tap=KC, dh=2)
                     for t in cwl_sb]

            HALF = N * BPC // 2

            def stage1(g):
                if 1 not in STAGES:
                    return
                for m in range(2):
                    ps = p_ps1.tile([128, BPC * S], F32, tag="ps1", name=f"ps1_{g}_{m}")
                    for kt in range(FKT):
                        base = (g % 4) * FKT * D + kt * D + m * 128
                        lhsT = w_sb[g // 4][:, base: base + 128]
                        rhs = x_sb[kt][:, g * S * BPC:(g + 1) * S * BPC]
                        nc.tensor.matmul(ps[:], lhsT, rhs,
                                         start=(kt == 0), stop=(kt == FKT - 1))
                    psv = ps[:].rearrange("p (n b) -> p n b", b=BPC)
                    hi = hh_w[:, m, g * S:(g + 1) * S, :]
                    lo = hl_w[:, m, g * S:(g + 1) * S, :]
                    if "hi" in PIECES:
                        nc.scalar.copy(hi, psv)                   # rounds to e4m3
                    if "lo" in PIECES:
                        nc.vector.scalar_tensor_tensor(           # lo = ps - hi
                            lo, psv, 1.0, hi, AluOpType.mult, AluOpType.subtract)

            def conv_tile(mo, bb, t0, nn):
                if 2 not in STAGES:
                    return
                ps = p_ps2.tile([128, 512], F32, tag="ps2", name=f"ps2_{mo}_{bb}_{t0}")
                ops = []
                for hv, wv, ntap in ((hh_w, cwh_v, KC), (hl_w, cwh_v, KC),
                                     (hh_w, cwl_v, CORR_TAPS)):
                    for tap in range(ntap):
                        # rhs [p, dh(2), nn]: dh stride N*BPC, n stride BPC
                        rhs = hv[:, :, t0 + tap: t0 + tap + nn, bb]
                        ops.append((wv[mo][:, tap], rhs))
                if "cmm" in PIECES:
                    for i, (l, r) in enumerate(ops):
                        nc.tensor.matmul(ps[:, :nn], l, r, start=(i == 0),
                                         stop=(i == len(ops) - 1),
                                         perf_mode=mybir.MatmulPerfMode.DoubleRow)
                if has_bias:
                    nc.vector.tensor_tensor(ps[:, :nn], ps[:, :nn],
                                            bc_sb[mo][:, t0:t0 + nn],
                                            AluOpType.add)
                y_sb = p_out.tile([128, 512], BF16, tag="y", name=f"y_{mo}_{bb}_{t0}")
                if "act" in PIECES:
                    nc.scalar.copy(y_sb[:, :nn], ps[:, :nn])  # ACT: psum -> bf16
                    nc.vector.scalar_tensor_tensor(           # LeakyReLU in-place
                        y_sb[:, :nn], y_sb[:, :nn], NEG_SLOPE, y_sb[:, :nn],
                        AluOpType.mult, AluOpType.max)
                if "dma" in PIECES:
                    nc.sync.dma_start(y_d[bb, mo * 128:(mo + 1) * 128, t0:t0 + nn],
                                      y_sb[:, :nn])

            # ---- DMA + compute emission (order == DMA priority) ----
            Q = N * BPC // 4

            def xq(q):                                # x quarter (groups 4q..4q+3)
                for kt in range(FKT):
                    nc.sync.dma_start(x_sb[kt][:, q * Q:(q + 1) * Q],
                                      xp_d[kt, :, q * Q:(q + 1) * Q])

            xq(0)
            nc.sync.dma_start(w_sb[0][:], w_d[0])     # w groups 0-3
            xq(1)
            nc.sync.dma_start(w_sb[1][:], w_d[1])     # w groups 4-7
            nc.sync.dma_start(cwh_sb[0][:], cwh_d[0])
            nc.sync.dma_start(cwl_sb[0][:], cwl_d[0])
            xq(2)
            nc.sync.dma_start(w_sb[2][:], w_d[2])     # w groups 8-11
            xq(3)
            nc.sync.dma_start(w_sb[3][:], w_d[3])     # w groups 12-15
            for mo in range(1, 4):
                nc.sync.dma_start(cwh_sb[mo][:], cwh_d[mo])
                nc.sync.dma_start(cwl_sb[mo][:], cwl_d[mo])
            if has_bias:
                for m in range(4):
                    nc.sync.dma_start(bc_sb[m][:], bc_d[m])

            # PE p-state warm-up: harmless tiny matmuls on a zeroed tile keep
            # the tensor engine busy (and its clock ramped) while the first
            # input DMAs land and across the stage-1 -> conv handoffs.
            warm_sb = p_x.tile([128, 64], BF16, tag="warm", name="warm_sb",
                               bufs=1)
            nc.vector.memset(warm_sb[:], 0)

            def pewarm(n, label):
                ps = p_ps1.tile([128, BPC * S], F32, tag="ps1",
                                name=f"warm_{label}")
                for i in range(n):
                    nc.tensor.matmul(ps[:64, :64], warm_sb[:, :64],
                                     warm_sb[:, :64], start=True, stop=True)

            pewarm(115, "boot")

            for g in range(8):                        # stage 1, groups 0-7
                stage1(g)

            pewarm(24, "bridge_a")                    # bridge the h-chain wait

            t0, nn = JT[0]                            # conv-A (needs groups 0-7)
            for mo in range(4):
                for bb in range(BPC):
                    conv_tile(mo, bb, t0, nn)

            for g in range(8, G):                     # stage 1, groups 8-15
                stage1(g)

            pewarm(24, "bridge_b")                    # bridge the h-chain wait

            t0, nn = JT[1]                            # conv-B (needs all groups)
            for mo in range(4):
                for bb in range(BPC):
                    if mo == 3 and bb == BPC - 1:
                        # split the final tile so the kernel tail (epilogue +
                        # store after the last matmul) is short
                        conv_tile(mo, bb, t0, 384)
                        conv_tile(mo, bb, t0 + 384, nn - 384)
                    else:
                        conv_tile(mo, bb, t0, nn)

    nc.compile()
    return nc


def kernel(x, idx, W, b, conv_w, conv_b):
    x = np.asarray(x); idx = np.asarray(idx); W = np.asarray(W)
    b = np.asarray(b); conv_w = np.asarray(conv_w); conv_b = np.asarray(conv_b)
    has_bias = bool(np.any(b) or np.any(conv_b))
    key = ("nc", has_bias)
    if key not in _cache:
        _cache[key] = _build(has_bias)
        _cache["nc"] = _cache[key]   # for test.py's TimelineSim hook
    nc = _cache[key]

    idx_flat = idx.reshape(-1).astype(np.int64)
    # x permuted + transposed: xp[c][kt, p, n, b] = x[4c+b, idx_flat[n], 128kt+p]
    xg = x[:, idx_flat, :].astype(BFnp)                       # [B, N, F]
    xp = np.ascontiguousarray(
        xg.transpose(2, 1, 0).reshape(FKT, 128, N, NCORES, BPC)
        .transpose(3, 0, 1, 2, 4)).reshape(NCORES, FKT, 128, N * BPC)

    # stage-1 weights, 4 groups per tile: wq[wt, p, gi, kt, d] = W[4wt+gi, 128kt+p, d]
    wq = np.ascontiguousarray(
        W.astype(BFnp).reshape(4, 4, FKT, 128, D).transpose(0, 3, 1, 2, 4)
    ).reshape(4, 128, 4 * FKT * D)

    # conv weights scaled into e4m3 range + e5m2 residual:
    # cw[mo, p, tap, dh, o] = conv_w[tap, 128dh+p, 128mo+o] * 64
    cws = (conv_w * np.float32(WSCALE)).reshape(KC, 2, 128, 4, 128)
    cws = np.ascontiguousarray(cws.transpose(3, 2, 0, 1, 4))  # [mo,p,tap,dh,o]
    cwh = cws.astype(E4np)
    cwl = (cws - cwh.astype(np.float32)).astype(E5np)
    cwh = cwh.reshape(4, 128, KC * 2 * 128)
    cwl = cwl.reshape(4, 128, KC * 2 * 128)

    in_maps = []
    for c in range(NCORES):
        m = {"xp": xp[c], "w": wq, "cwh": cwh, "cwl": cwl}
        if has_bias:
            # bias_conv[t, o] = sum_tap b[g(t+tap)] @ conv_w[tap] + conv_b
            M = np.einsum('gd,tdo->gto', b, conv_w, optimize=True)  # [G, KC, O]
            P = np.concatenate([np.zeros((G, 1, O), np.float32),
                                np.cumsum(M, axis=1)], axis=1)      # [G, KC+1, O]
            t = np.arange(T)
            q, r = t >> 6, t & 63
            j1 = np.minimum(64 - r, KC)
            bc = P[q, j1] + (P[np.minimum(q + 1, G - 1), KC]
                             - P[np.minimum(q + 1, G - 1), j1]) * (j1 < KC)[:, None]
            bc = (bc + conv_b[None, :]) * np.float32(WSCALE)        # [T, O]
            m["bc"] = np.ascontiguousarray(
                bc.T.reshape(4, 128, T)).astype(np.float32)
        in_maps.append(m)

    res = run_bass_kernel_spmd(nc, in_maps, core_ids=list(range(NCORES)),
                               trace=TRACE)
    if TRACE and res.exec_time_ns is not None:
        print(f"HW exec time: {res.exec_time_ns} ns")
        if res.instructions_and_trace is not None:
            print("trace:", res.instructions_and_trace[1])
    y = np.stack([r["y"] for r in res.results])       # [NC, BPC, O, T] bf16
    y = y.reshape(B, O, T).transpose(0, 2, 1).astype(np.float32)
    return np.ascontiguousarray(y * np.float32(1.0 / WSCALE))
